# revision 23
# baseline (speedup 1.0000x reference)
"""Trainium2 Bass kernel for nn_MixedAttention (ConvBERT-style mixed attention).

Sharding: data-parallel over (batch=4) x (seq halves=2) = 8 cores.
Each core computes output rows [j*1024, (j+1)*1024) of batch b, core = 2*b + j.
k/v are computed redundantly on both cores of a batch pair (no collectives).

v2 design (vs. baseline):
  - Outputs ship in producer layout and the host finishes the math: attention
    context goes out as [65, seq] PSUM tiles per head (row 64 = softmax
    denominator via an appended ones column on v), the conv branch goes out as
    [a, seq] plus the raw span-weight numerators (pck); host divides and
    transposes.  This removes every PE transpose, PSUM->SBUF evac copy,
    reciprocal and staging multiply of the baseline's ~100us tail.
  - The conv span-weight chain (depthwise conv, pointwise conv, conv-kernel
    layer) runs in fp8e4 with DoubleRow perf mode (2 contraction tiles per
    matmul = 2x PE throughput).  Host scales those weights x32 so fp8 normals
    cover them; the exp activation descales via its scale argument.  Noise in
    this chain is squashed by the span softmax (logits are ~1e-3), so fp8 is
    numerically free here.
  - Inputs stream per-dh-tile across several DMA queues so the first
    projection matmul issues ~2us in instead of ~25us.
  - Emission interleaves projection/conv matmuls into the attention phase gap
    so the PE stays continuously busy (pstate stays at 2.4 GHz).
"""

import sys

for _p in ("/opt/trn_rl_repo",):
    if _p not in sys.path:
        sys.path.insert(0, _p)

import numpy as np
import ml_dtypes

HIDDEN = 768
N_HEADS = 6
HEAD_DIM = 64
ALL_HEAD = 384
K = 9
B, S = 4, 2048
CHUNK = 1024          # seq rows per core
N_CORES = 8
BF16 = ml_dtypes.bfloat16
FP8 = ml_dtypes.float8_e4m3

W8SCALE = 32.0        # host premultiplier for fp8-stored weights
CASCALE = 64.0        # device premultiplier for conv_attn before fp8 store

_COMPILED = {}


def _build_program():
    import concourse.bass as bass
    import concourse.mybir as mybir
    import concourse.tile as tile
    from concourse import bacc
    from contextlib import ExitStack

    dt = mybir.dt
    Alu = mybir.AluOpType
    Act = mybir.ActivationFunctionType
    DR = mybir.MatmulPerfMode.DoubleRow

    nc = bacc.Bacc("TRN2", target_bir_lowering=False, debug=False)

    def din(name, shape, dtype=dt.bfloat16):
        return nc.dram_tensor(name, list(shape), dtype, kind="ExternalInput").ap()

    x_full = din("x_full", [128, 6 * S])               # xT full batch [c, s]
    x_loc = din("x_loc", [128, 6 * 1032])              # xT chunk+-4 (zero pad)
    wq = din("wq", [128, 6 * ALL_HEAD])
    wk = din("wk", [128, 6 * ALL_HEAD])
    wv = din("wv", [128, 6 * ALL_HEAD])
    wco = din("wco", [128, 6 * ALL_HEAD])
    wpw8 = din("wpw8", [128, 6 * ALL_HEAD], dt.float8e4)   # pw.T * 32
    wck8 = din("wck8", [128, 3 * 64], dt.float8e4)   # Wck.T * 32, padded to 64
    dwd8 = din("dwd8", [128, 6 * 5 * 2 * 128], dt.float8e4)  # diag dw mats * 32
    bvrow = din("bvrow", [1, ALL_HEAD])
    comask = din("comask", [1, 1032])
    bq = din("bq", [128, 3], dt.float32)
    bk = din("bk", [128, 3], dt.float32)
    convb = din("convb", [128, 3], dt.float32)
    bco = din("bco", [128, 3], dt.float32)
    bck = din("bck", [54, 1], dt.float32)

    out_attn = nc.dram_tensor("out_attn", [65, 6 * CHUNK], dt.float32,
                              kind="ExternalOutput").ap()
    out_conv = nc.dram_tensor("out_conv", [128, 3 * CHUNK], dt.bfloat16,
                              kind="ExternalOutput").ap()
    pck_dram = nc.dram_tensor("pck_out", [54, CHUNK], dt.bfloat16,
                              kind="ExternalOutput").ap()

    with tile.TileContext(nc) as tc, ExitStack() as ctx:
        singles = ctx.enter_context(tc.tile_pool(name="singles", bufs=1))
        persist = ctx.enter_context(tc.tile_pool(name="persist", bufs=1))
        work = ctx.enter_context(tc.tile_pool(name="work", bufs=3))

        # ---------------- SBUF destination tiles for inputs ----------------
        # x_full split into two sb-block tiles so k/v matmuls on the first
        # 1024 columns start as soon as the first 1.5MB DMA lands.
        xsb0 = singles.tile([128, 6, 1024], dt.bfloat16, name="xsb0")
        xsb1 = singles.tile([128, 6, 1024], dt.bfloat16, name="xsb1")
        xlsb = singles.tile([128, 6, 1032], dt.bfloat16, name="xlsb")
        xl8 = singles.tile([128, 6, 1032], dt.float8e4, name="xl8")
        wq_sb = singles.tile([128, 6, ALL_HEAD], dt.bfloat16, name="wq_sb")
        wk_sb = singles.tile([128, 6, ALL_HEAD], dt.bfloat16, name="wk_sb")
        wv_sb = singles.tile([128, 6, ALL_HEAD], dt.bfloat16, name="wv_sb")
        wco_sb = singles.tile([128, 6, ALL_HEAD], dt.bfloat16, name="wco_sb")
        wpw_sb = singles.tile([128, 6, ALL_HEAD], dt.float8e4, name="wpw_sb")
        wck_sb = singles.tile([128, 3, 64], dt.float8e4, name="wck_sb")
        dwd_sb = singles.tile([128, 6, 5, 2, 128], dt.float8e4, name="dwd_sb")
        bv_sb = singles.tile([1, ALL_HEAD], dt.bfloat16, name="bv_sb")
        bq_sb = singles.tile([128, 3], dt.float32, name="bq_sb")
        bk_sb = singles.tile([128, 3], dt.float32, name="bk_sb")
        convb_sb = singles.tile([128, 3], dt.float32, name="convb_sb")
        bco_sb = singles.tile([128, 3], dt.float32, name="bco_sb")
        bck_sb = singles.tile([54, 1], dt.float32, name="bck_sb")
        mask_sb = singles.tile([128, 1032], dt.bfloat16, name="mask_sb")

        # DMA queues: only SP (sync), Activation (scalar) and gpsimd can
        # issue DMAs.  Few big transfers beat many chunks (each issue costs
        # ~0.6-1.3us of queue time).  Critical path: wk+xsb0 (k), wq+xlsb
        # (q); everything else trails.
        nc.scalar.dma_start(out=wq_sb, in_=wq)
        nc.scalar.dma_start(out=wk_sb, in_=wk)
        nc.scalar.dma_start(out=bq_sb, in_=bq)
        nc.scalar.dma_start(out=bk_sb, in_=bk)
        nc.scalar.dma_start(out=xlsb, in_=x_loc)
        nc.scalar.dma_start(out=wco_sb, in_=wco)
        nc.scalar.dma_start(out=convb_sb, in_=convb)
        nc.scalar.dma_start(out=bco_sb, in_=bco)
        nc.scalar.dma_start(out=bck_sb, in_=bck)
        for dh in range(6):
            nc.sync.dma_start(out=xsb0[:, dh, :],
                              in_=x_full[:, dh * S: dh * S + 1024])
        for dh in range(6):
            nc.sync.dma_start(out=xsb1[:, dh, :],
                              in_=x_full[:, dh * S + 1024:(dh + 1) * S])
        nc.gpsimd.dma_start(out=wv_sb, in_=wv)
        nc.gpsimd.dma_start(out=bv_sb, in_=bvrow)
        nc.gpsimd.dma_start(out=wpw_sb, in_=wpw8)
        nc.gpsimd.dma_start(out=wck_sb, in_=wck8)
        nc.gpsimd.dma_start(out=mask_sb, in_=comask.to_broadcast([128, 1032]))
        nc.gpsimd.dma_start(out=dwd_sb, in_=dwd8)
        # x_loc8 is derived on-device (saves 0.8MB of startup DMA)
        for dh in range(6):
            nc.vector.tensor_copy(xl8[:, dh, :], xlsb[:, dh, :])

        ones_sb = singles.tile([1, 128], dt.bfloat16, name="ones_sb")
        nc.vector.memset(ones_sb, 1.0)

        # persistent intermediates
        qT = persist.tile([128, 3, CHUNK], dt.bfloat16, name="qT")
        kT = persist.tile([128, 3, S], dt.bfloat16, name="kT")
        dwT = persist.tile([128, 6, CHUNK], dt.float8e4, name="dwT")  # 32*dw_out
        kcT = persist.tile([128, 3, CHUNK], dt.bfloat16, name="kcT")
        caT = persist.tile([128, 3, CHUNK], dt.float8e4, name="caT")  # 64*ca
        coT = persist.tile([128, 3, 1032], dt.bfloat16, name="coT")
        vsb = persist.tile([128, 16, 6, 65], dt.bfloat16, name="vsb")
        nc.vector.memset(vsb[:, :, :, 64:65], 1.0)
        pck = persist.tile([54, CHUNK], dt.bfloat16, name="pck")
        acc3 = persist.tile([128, 3, CHUNK], dt.bfloat16, name="acc3")

        pj = ctx.enter_context(tc.tile_pool(name="psum_pj", bufs=2,
                                            space="PSUM"))
        pa = ctx.enter_context(tc.tile_pool(name="psum_sc", bufs=2,
                                            space="PSUM"))
        pc = ctx.enter_context(tc.tile_pool(name="psum_ctx", bufs=1,
                                            space="PSUM"))

        # ---------------- filler emitters (PE work interleaved into the
        # attention phase; list order respects producer dependencies) -------
        def q_at(at):
            def emit():
                for sb in range(2):
                    ps = pj.tile([128, 512], dt.float32, tag="pj", name="psq")
                    for dh in range(6):
                        nc.tensor.matmul(
                            ps, wq_sb[:, dh, at * 128:(at + 1) * 128],
                            xlsb[:, dh, 4 + sb * 512: 4 + (sb + 1) * 512],
                            start=(dh == 0), stop=(dh == 5))
                    nc.vector.tensor_scalar_add(
                        qT[:, at, sb * 512:(sb + 1) * 512], ps,
                        bq_sb[:, at:at + 1])
            return emit

        def k_at(at, sb):
            def emit():
                xs = xsb0 if sb < 2 else xsb1
                o = (sb % 2) * 512
                ps = pj.tile([128, 512], dt.float32, tag="pj", name="psk")
                for dh in range(6):
                    nc.tensor.matmul(
                        ps, wk_sb[:, dh, at * 128:(at + 1) * 128],
                        xs[:, dh, o:o + 512],
                        start=(dh == 0), stop=(dh == 5))
                nc.vector.tensor_scalar_add(
                    kT[:, at, sb * 512:(sb + 1) * 512], ps, bk_sb[:, at:at + 1])
            return emit

        def v_st(st):
            def emit():
                xs = xsb0 if st < 8 else xsb1
                o = (st % 8) * 128
                pvf = pj.tile([128, 512], dt.float32, tag="pj", name="psv")
                pv = pvf[:, 0:ALL_HEAD]
                for dh in range(6):
                    nc.tensor.matmul(
                        pv, xs[:, dh, o:o + 128],
                        wv_sb[:, dh, :], start=(dh == 0), stop=False)
                nc.tensor.matmul(pv, ones_sb, bv_sb, start=False, stop=True)
                nc.vector.tensor_copy(vsb[:, st, :, 0:64], pv.rearrange(
                    "p (h d) -> p h d", h=6))
            return emit

        def dw_ct(ct, sb):
            def emit():
                pdw = pj.tile([128, 512], dt.float32, tag="pj", name="psd")
                for kp in range(4):      # tap pairs (0,1)..(6,7), DoubleRow
                    base = xl8[:, ct, 2 * kp + sb * 512: 2 * kp + sb * 512 + 1]
                    rhs = bass.AP(
                        tensor=xl8.tensor, offset=base.offset,
                        ap=[list(base.ap[0]), [1, 2], [1, 512]])
                    nc.tensor.matmul(
                        pdw, dwd_sb[:, ct, kp, :, :], rhs,
                        start=(kp == 0), stop=False, perf_mode=DR)
                nc.tensor.matmul(      # tap 8, plain fp8
                    pdw, dwd_sb[:, ct, 4, 0, :],
                    xl8[:, ct, 8 + sb * 512: 8 + sb * 512 + 512],
                    start=False, stop=True)
                nc.vector.tensor_copy(dwT[:, ct, sb * 512:(sb + 1) * 512], pdw)
            return emit

        def pw_at(at, sb):
            def emit():
                pp = pj.tile([128, 512], dt.float32, tag="pj", name="psp")
                for dp in range(3):      # ct pairs, DoubleRow
                    nc.tensor.matmul(
                        pp, wpw_sb[:, 2 * dp:2 * dp + 2,
                                   at * 128:(at + 1) * 128],
                        dwT[:, 2 * dp:2 * dp + 2, sb * 512:(sb + 1) * 512],
                        start=(dp == 0), stop=(dp == 2), perf_mode=DR)
                # psum = 1024*kc ; evac to true-scale kc + conv bias
                nc.vector.tensor_scalar(
                    out=kcT[:, at, sb * 512:(sb + 1) * 512], in0=pp,
                    scalar1=1.0 / 1024.0, scalar2=convb_sb[:, at:at + 1],
                    op0=Alu.mult, op1=Alu.add)
            return emit

        def ca_at(at):
            def emit():
                nc.vector.scalar_tensor_tensor(
                    out=caT[:, at, :], in0=kcT[:, at, :], scalar=CASCALE,
                    in1=qT[:, at, :], op0=Alu.mult, op1=Alu.mult)
            return emit

        def ckl_sb(sb):
            def emit():
                pkf = pj.tile([128, 512], dt.float32, tag="pj", name="psl")
                pk = pkf[0:54, :]
                nc.tensor.matmul(
                    pk, wck_sb[:, 0:2, 0:54],
                    caT[:, 0:2, sb * 512:(sb + 1) * 512],
                    start=True, stop=False, perf_mode=DR)
                nc.tensor.matmul(
                    pk, wck_sb[:, 2, 0:54],
                    caT[:, 2, sb * 512:(sb + 1) * 512],
                    start=False, stop=True)
                # psum = W8SCALE*CASCALE * ckl
                nc.scalar.activation(pck[:, sb * 512:(sb + 1) * 512], pk,
                                     Act.Exp, bias=bck_sb,
                                     scale=1.0 / (W8SCALE * CASCALE))
            return emit

        def pck_out():
            def emit():
                nc.scalar.dma_start(out=pck_dram, in_=pck)
            return emit

        def co_at(at, blk):
            def emit():
                o, w = blk
                pco = pj.tile([128, 512], dt.float32, tag="pj", name="psc")
                for dh in range(6):
                    nc.tensor.matmul(
                        pco[:, :w], wco_sb[:, dh, at * 128:(at + 1) * 128],
                        xlsb[:, dh, o:o + w],
                        start=(dh == 0), stop=(dh == 5))
                nc.vector.scalar_tensor_tensor(
                    out=coT[:, at, o:o + w], in0=pco[:, :w],
                    scalar=bco_sb[:, at:at + 1], in1=mask_sb[:, o:o + w],
                    op0=Alu.add, op1=Alu.mult)
            return emit

        # conv window einsum, one tap per filler: 2 coalesced broadcast DMAs
        # (sync queue -- gpsimd DMA issue costs ~1.3us each and starved the
        # whole Pool engine in v3), mul on gpsimd, serial add chain on DVE.
        def einsum_k(k):
            def emit():
                ckb = work.tile([128, 3, CHUNK], dt.bfloat16, tag="ckb",
                                bufs=2, name="ckb")
                for hh in range(2):
                    srcap = bass.AP(
                        tensor=pck_dram.tensor,
                        offset=(9 * hh + k) * CHUNK,
                        ap=[[0, 64], [18 * CHUNK, 3], [1, CHUNK]])
                    nc.sync.dma_start(
                        out=ckb[hh * 64:(hh + 1) * 64, :, :], in_=srcap)
                cob = coT[:, 0, k:k + 1]
                cosrc = bass.AP(
                    tensor=coT.tensor, offset=cob.offset,
                    ap=[list(cob.ap[0]), [1032, 3], [1, CHUNK]])
                if k == 0:
                    nc.gpsimd.tensor_mul(acc3, ckb, cosrc)
                else:
                    tmp = work.tile([128, 3, CHUNK], dt.bfloat16, tag="tmp",
                                    bufs=2, name="tmp")
                    nc.gpsimd.tensor_mul(tmp, ckb, cosrc)
                    nc.vector.tensor_add(acc3, acc3, tmp)
                if k == K - 1:
                    nc.gpsimd.dma_start(out=out_conv, in_=acc3)
            return emit

        fillers = []
        fillers += [v_st(st) for st in range(16)]
        fillers += [q_at(1), k_at(1, 0), k_at(1, 1), k_at(1, 2), k_at(1, 3)]
        fillers += [dw_ct(ct, sb) for ct in range(6) for sb in range(2)]
        fillers += [q_at(2)]
        fillers += [pw_at(at, sb) for at in range(3) for sb in range(2)]
        fillers += [co_at(at, blk) for at in range(3)
                    for blk in ((0, 512), (512, 512), (1024, 8))]
        fillers += [ca_at(at) for at in range(3)]
        fillers += [ckl_sb(sb) for sb in range(2)]
        fillers += [pck_out()]
        fillers += [einsum_k(k) for k in range(K)]
        fillers += [k_at(2, sb) for sb in range(4)]

        # ---------------- attention: flat one-step software pipeline -------
        # PE order per step i: scores(i), [fillers], ctx(i-1).  ctx(i-1)
        # waits on exp(i-1), so putting scores(i) (and filler) ahead of it
        # keeps the PE busy while ACT runs and lets exp(i) start the moment
        # exp(i-1) finishes: the ACT exp stream runs back-to-back instead of
        # serializing with the PE (which cost ~850ns/step in v2).
        q_at(0)()
        for sb in range(4):
            k_at(0, sb)()
        fillers.pop(0)()          # v_st(0) ahead of ctx(h0, 0)

        steps = [(h, sk) for h in range(N_HEADS) for sk in range(16)]
        cps_of = {}
        prev = None               # (h, sk, pt) awaiting its ctx matmuls
        n_fill0 = len(fillers)
        fill_done = 0

        def emit_ctx(h, sk, pt):
            for sb in range(2):
                nc.tensor.matmul(
                    cps_of[h][sb], vsb[:, sk, h, :],
                    pt[:, sb * 512:(sb + 1) * 512],
                    start=(sk == 0), stop=(sk == 15))
            if sk == 15:
                for sb in range(2):
                    cstg = work.tile([65, 512], dt.float32, tag="cstg",
                                     bufs=4, name="cstg")
                    nc.vector.tensor_copy(cstg, cps_of[h][sb])
                    nc.sync.dma_start(
                        out=out_attn[:, h * CHUNK + sb * 512:
                                     h * CHUNK + (sb + 1) * 512],
                        in_=cstg)

        for i, (h, sk) in enumerate(steps):
            at, lo = h // 2, (h % 2) * 64
            if sk == 0:
                cps_of[h] = [pc.tile([65, 512], dt.float32, tag=f"ctx{sb}",
                                     name=f"cps{sb}") for sb in range(2)]
            sc = pa.tile([128, 1024], dt.float32, tag="sc", name="sc")
            for sb in range(2):
                nc.tensor.matmul(
                    sc[:, sb * 512:(sb + 1) * 512],
                    kT[lo:lo + 64, at, sk * 128:(sk + 1) * 128],
                    qT[lo:lo + 64, at, sb * 512:(sb + 1) * 512],
                    start=True, stop=True)
            # pace fillers: one per step while v tiles stream (steps 0-15),
            # then spread the rest so they finish around step 64
            target = i + 2 if i < 16 else 18 + int((i - 15) * (n_fill0 - 18) / 48.0)
            while fill_done < min(target, n_fill0) and fillers:
                fillers.pop(0)()
                fill_done += 1
            pt = work.tile([128, 1024], dt.bfloat16, tag="pt", bufs=3,
                           name="pt")
            nc.scalar.activation(pt, sc, Act.Exp, scale=0.125)
            if prev is not None:
                emit_ctx(*prev)
            prev = (h, sk, pt)
        emit_ctx(*prev)

    nc.compile()
    return nc


def _prep_in_maps(inputs):
    x = np.asarray(inputs["x"], np.float32)
    dw = np.asarray(inputs["dw"], np.float32).reshape(HIDDEN, K)

    def sb_layout(wT, ntile):  # [ntile*128, F] -> [128, ntile*F]
        f = wT.shape[1]
        return np.ascontiguousarray(
            wT.reshape(ntile, 128, f).transpose(1, 0, 2).reshape(128, ntile * f))

    def wprep(w, dtype=BF16, scale=1.0):  # [A, HIDDEN] -> [128, 6*A]
        return sb_layout(np.ascontiguousarray(w.T * scale).astype(dtype), 6)

    com = {
        "wq": wprep(inputs["Wq"]), "wk": wprep(inputs["Wk"]),
        "wv": wprep(inputs["Wv"]), "wco": wprep(inputs["Wco"]),
        "wpw8": wprep(inputs["pw"], FP8, W8SCALE),
        "wck8": sb_layout(np.pad(
            np.ascontiguousarray(inputs["Wck"].T * W8SCALE),
            ((0, 0), (0, 10))).astype(FP8), 3),
        "bvrow": inputs["bv"].reshape(1, ALL_HEAD).astype(BF16),
        "bq": np.ascontiguousarray(inputs["bq"].reshape(3, 128).T, np.float32),
        "bk": np.ascontiguousarray(inputs["bk"].reshape(3, 128).T, np.float32),
        "convb": np.ascontiguousarray(
            inputs["conv_bias"].reshape(3, 128).T, np.float32),
        "bco": np.ascontiguousarray(inputs["bco"].reshape(3, 128).T, np.float32),
        "bck": inputs["bck"].reshape(54, 1).astype(np.float32),
    }
    # diagonal depthwise matrices (x32): dwd[c', ct, kp, i, c] for tap 2kp+i
    dwdm = np.zeros((128, 6, 5, 2, 128), FP8)
    ii = np.arange(128)
    for ct in range(6):
        for k in range(K):
            dwdm[ii, ct, k // 2, k % 2, ii] = (
                dw[ct * 128 + ii, k] * W8SCALE).astype(FP8)
    com["dwd8"] = dwdm.reshape(128, 6 * 5 * 2 * 128)

    in_maps = []
    for b in range(B):
        xb = x[b]                                   # [S, HIDDEN]
        xTb = np.ascontiguousarray(xb.T)            # [768, S] fp32
        xT_pad = np.zeros((HIDDEN, S + 8), np.float32)
        xT_pad[:, 4:4 + S] = xTb
        for j in range(2):
            loc = np.ascontiguousarray(xT_pad[:, j * CHUNK: j * CHUNK + 1032])
            g0 = j * CHUNK - 4
            mrows = np.arange(g0, g0 + 1032)
            comask = ((mrows >= 0) & (mrows < S)).astype(BF16).reshape(1, 1032)
            m = dict(com)
            m["x_full"] = sb_layout(xTb.astype(BF16), 6)
            m["x_loc"] = sb_layout(loc.astype(BF16), 6)
            m["comask"] = comask
            in_maps.append(m)
    return in_maps


def _gather_core(r):
    # attention: [65, 6*1024] fp32, row 64 = softmax denominator
    att = np.asarray(r["out_attn"], np.float32).reshape(65, 6, CHUNK)
    ctx = att[0:64] / att[64:65]                       # [64, 6, s]
    ctx = ctx.transpose(2, 1, 0).reshape(CHUNK, ALL_HEAD)
    # conv: [128, 3*1024] bf16 numerators / pck-sum denominators
    cnv = np.asarray(r["out_conv"], np.float32).reshape(128, 3, CHUNK)
    cnv = cnv.transpose(1, 0, 2).reshape(ALL_HEAD, CHUNK)  # [a, s]
    pck = np.asarray(r["pck_out"], np.float32).reshape(6, K, CHUNK)
    den = pck.sum(axis=1)                              # [h, s]
    cnv = cnv.reshape(N_HEADS, HEAD_DIM, CHUNK) / den[:, None, :]
    cnv = cnv.reshape(ALL_HEAD, CHUNK).T               # [s, a]
    return np.concatenate([ctx, cnv], axis=1)          # [1024, 768]


def _gather(results):
    outs = [_gather_core(r) for r in results]
    full = np.stack(outs).reshape(B, 2, CHUNK, 768).reshape(B, S, 768)
    return np.ascontiguousarray(full, np.float32)


def kernel(**inputs):
    from concourse.bass_utils import run_bass_kernel_spmd

    key = "prog"
    if key not in _COMPILED:
        _COMPILED[key] = _build_program()
    nc = _COMPILED[key]
    in_maps = _prep_in_maps(inputs)
    res = run_bass_kernel_spmd(nc, in_maps, list(range(N_CORES)))
    return _gather(res.results)


if __name__ == "__main__":
    import reference
    inp = {k: np.asarray(v) for k, v in reference.setup_inputs().items()}
    got = kernel(**inp)
    want = np.asarray(reference.reference(**inp))
    err = np.linalg.norm(got - want) / np.linalg.norm(want)
    print("rel err:", err)


# revision 25
# speedup vs baseline: 1.0431x; 1.0431x over previous
"""Trainium2 Bass kernel for nn_MixedAttention (ConvBERT-style mixed attention).

Sharding: data-parallel over (batch=4) x (seq halves=2) = 8 cores.
Each core computes output rows [j*1024, (j+1)*1024) of batch b, core = 2*b + j.
k/v are computed redundantly on both cores of a batch pair (no collectives).

v5 design notes:
  - Attention is key-permutation invariant, so each core gets x ROTATED so its
    chunk (+/-4 halo for the conv) sits at fixed columns: x0 = rotated cols
    [0,1036) with OOB halo columns zeroed, x1 = rotated cols [1036,2060).
    One x buffer serves q/k/v/conv; total input drops to ~6MB in 5 DMAs.
  - Inputs are packed into mega-tensors (megaA/megaB/mega8/megaF) because
    every [128,*] DMA costs ~128 descriptor-packets regardless of size.
  - Outputs ship in producer layout; host normalizes and transposes (row 64
    of each attention tile is the softmax denominator from an appended ones
    column on v; conv denominators come from summing the shipped pck).
  - The conv span-weight chain (depthwise, pointwise, conv-kernel layer) runs
    in fp8e4 DoubleRow (2 contraction tiles/matmul = 2x PE).  Host scales
    those weights x32 into fp8 normal range; descaled inside the exp.  The
    span softmax logits are ~1e-3 so fp8 noise vanishes after softmax.
  - The span-weight broadcast (pck row -> 64 head partitions) is done with
    0/1 selector matmuls on the PE instead of 7MB of stride-0 DMA.
  - Emission is a flat one-step software pipeline over (head, sk) steps:
    scores(i) -> fillers -> exp(i) on ACT -> ctx(i-1), so the ACT exp stream
    runs back-to-back while projection/conv matmuls fill the PE gap.
"""

import sys

for _p in ("/opt/trn_rl_repo",):
    if _p not in sys.path:
        sys.path.insert(0, _p)

import numpy as np
import ml_dtypes

HIDDEN = 768
N_HEADS = 6
HEAD_DIM = 64
ALL_HEAD = 384
K = 9
B, S = 4, 2048
CHUNK = 1024          # seq rows per core
N_CORES = 8
BF16 = ml_dtypes.bfloat16
FP8 = ml_dtypes.float8_e4m3

W8SCALE = 32.0        # host premultiplier for fp8-stored weights
CASCALE = 64.0        # device premultiplier for conv_attn before fp8 store

# mega-tensor column maps (bf16 A/B, fp8, fp32)
A_WQ, A_WK = 0, 2304
A_COLS = 4608
B_WV, B_WO, B_MASK, B_BV, B_SEL = 0, 2304, 4608, 5640, 6144
B_COLS = 6144 + 27 * 128
F8_PW, F8_CK, F8_DW = 0, 2304, 2496
F8_COLS = 2496 + 7680
F_BQ, F_BK, F_CB, F_BO, F_BCK = 0, 3, 6, 9, 12
F_COLS = 13

_COMPILED = {}


def _build_program():
    import concourse.bass as bass
    import concourse.mybir as mybir
    import concourse.tile as tile
    from concourse import bacc
    from contextlib import ExitStack

    dt = mybir.dt
    Alu = mybir.AluOpType
    Act = mybir.ActivationFunctionType
    DR = mybir.MatmulPerfMode.DoubleRow

    nc = bacc.Bacc("TRN2", target_bir_lowering=False, debug=False)

    def din(name, shape, dtype=dt.bfloat16):
        return nc.dram_tensor(name, list(shape), dtype, kind="ExternalInput").ap()

    x0d = din("x0", [128, 6 * 1036])
    x1d = din("x1", [128, 6 * 1024])
    megaA = din("megaA", [128, A_COLS])
    megaB = din("megaB", [128, B_COLS])
    mega8 = din("mega8", [128, F8_COLS], dt.float8e4)
    megaF = din("megaF", [128, F_COLS], dt.float32)

    out_attn = nc.dram_tensor("out_attn", [65, 6 * CHUNK], dt.float32,
                              kind="ExternalOutput").ap()
    out_conv = nc.dram_tensor("out_conv", [128, 3 * CHUNK], dt.bfloat16,
                              kind="ExternalOutput").ap()
    pck_dram = nc.dram_tensor("pck_out", [54, CHUNK], dt.bfloat16,
                              kind="ExternalOutput").ap()

    with tile.TileContext(nc) as tc, ExitStack() as ctx:
        singles = ctx.enter_context(tc.tile_pool(name="singles", bufs=1))
        persist = ctx.enter_context(tc.tile_pool(name="persist", bufs=1))
        work = ctx.enter_context(tc.tile_pool(name="work", bufs=3))

        # ---------------- SBUF input tiles + DMAs ----------------
        x0 = singles.tile([128, 6, 1036], dt.bfloat16, name="x0")
        x1 = singles.tile([128, 6, 1024], dt.bfloat16, name="x1")
        mA = singles.tile([128, A_COLS], dt.bfloat16, name="mA")
        mB = singles.tile([128, B_COLS], dt.bfloat16, name="mB")
        m8 = singles.tile([128, F8_COLS], dt.float8e4, name="m8")
        mF = singles.tile([128, F_COLS], dt.float32, name="mF")
        xl8 = singles.tile([128, 6, 1032], dt.float8e4, name="xl8")

        nc.scalar.dma_start(out=mA, in_=megaA)
        nc.sync.dma_start(out=x0, in_=x0d)
        nc.gpsimd.dma_start(out=m8, in_=mega8)
        nc.gpsimd.dma_start(out=mF, in_=megaF)
        nc.sync.dma_start(out=x1, in_=x1d)
        nc.scalar.dma_start(out=mB, in_=megaB)

        # weight / constant views into the megas
        def view3(t, col, n, w):
            return t[:, col:col + n * w].rearrange("p (n w) -> p n w", n=n)

        wq_sb = view3(mA, A_WQ, 6, ALL_HEAD)
        wk_sb = view3(mA, A_WK, 6, ALL_HEAD)
        wv_sb = view3(mB, B_WV, 6, ALL_HEAD)
        wco_sb = view3(mB, B_WO, 6, ALL_HEAD)
        mask_sb = mB[:, B_MASK:B_MASK + 1032]
        bv_sb = mB[0:1, B_BV:B_BV + ALL_HEAD]
        sel_sb = view3(mB, B_SEL, 27, 128)[0:54]        # [54, 27, 128]
        wpw_sb = view3(m8, F8_PW, 6, ALL_HEAD)
        wck_sb = view3(m8, F8_CK, 3, 64)
        dwd_sb = view3(m8, F8_DW, 30, 256).rearrange(
            "p (ct kp) w -> p ct kp w", ct=6)           # [128, 6, 5, 256]
        bq_sb = mF[:, F_BQ:F_BQ + 3]
        bk_sb = mF[:, F_BK:F_BK + 3]
        convb_sb = mF[:, F_CB:F_CB + 3]
        bco_sb = mF[:, F_BO:F_BO + 3]
        bck_sb = mF[0:54, F_BCK:F_BCK + 1]

        ones_sb = singles.tile([1, 128], dt.bfloat16, name="ones_sb")
        nc.vector.memset(ones_sb, 1.0)

        # x_loc8 view (fp8 copy of x0 cols [4,1036) = chunk +-4 halo)
        for dh in range(6):
            nc.vector.tensor_copy(xl8[:, dh, :], x0[:, dh, 4:1036])

        # persistent intermediates
        qT = persist.tile([128, 3, CHUNK], dt.bfloat16, name="qT")
        kT = persist.tile([128, 3, S], dt.bfloat16, name="kT")
        dwT = persist.tile([128, 6, CHUNK], dt.float8e4, name="dwT")  # 32*dw
        kcT = persist.tile([128, 3, CHUNK], dt.bfloat16, name="kcT")
        caT = persist.tile([128, 3, CHUNK], dt.float8e4, name="caT")  # 64*ca
        coT = persist.tile([128, 3, 1032], dt.bfloat16, name="coT")
        vsb = persist.tile([128, 16, 6, 65], dt.bfloat16, name="vsb")
        nc.vector.memset(vsb[:, :, :, 64:65], 1.0)
        pck = persist.tile([54, CHUNK], dt.bfloat16, name="pck")
        acc3 = persist.tile([128, 3, CHUNK], dt.bfloat16, name="acc3")

        pj = ctx.enter_context(tc.tile_pool(name="psum_pj", bufs=2,
                                            space="PSUM"))
        pa = ctx.enter_context(tc.tile_pool(name="psum_sc", bufs=2,
                                            space="PSUM"))
        pc = ctx.enter_context(tc.tile_pool(name="psum_ctx", bufs=1,
                                            space="PSUM"))

        # ---------------- filler emitters (producer-ordered) ----------------
        def q_at(at):
            def emit():
                for sb in range(2):
                    ps = pj.tile([128, 512], dt.float32, tag="pj", name="psq")
                    for dh in range(6):
                        nc.tensor.matmul(
                            ps, wq_sb[:, dh, at * 128:(at + 1) * 128],
                            x0[:, dh, 8 + sb * 512: 8 + (sb + 1) * 512],
                            start=(dh == 0), stop=(dh == 5))
                    nc.vector.tensor_scalar_add(
                        qT[:, at, sb * 512:(sb + 1) * 512], ps,
                        bq_sb[:, at:at + 1])
            return emit

        def k_at(at, sb):
            def emit():
                xs, o = (x0, 8 + sb * 512) if sb < 2 else (x1, (sb - 2) * 512)
                ps = pj.tile([128, 512], dt.float32, tag="pj", name="psk")
                for dh in range(6):
                    nc.tensor.matmul(
                        ps, wk_sb[:, dh, at * 128:(at + 1) * 128],
                        xs[:, dh, o:o + 512],
                        start=(dh == 0), stop=(dh == 5))
                nc.vector.tensor_scalar_add(
                    kT[:, at, sb * 512:(sb + 1) * 512], ps, bk_sb[:, at:at + 1])
            return emit

        def v_st(st):
            def emit():
                xs, o = (x0, 8 + st * 128) if st < 8 else (x1, (st - 8) * 128)
                pvf = pj.tile([128, 512], dt.float32, tag="pj", name="psv")
                pv = pvf[:, 0:ALL_HEAD]
                for dh in range(6):
                    nc.tensor.matmul(
                        pv, xs[:, dh, o:o + 128],
                        wv_sb[:, dh, :], start=(dh == 0), stop=False)
                nc.tensor.matmul(pv, ones_sb, bv_sb, start=False, stop=True)
                nc.vector.tensor_copy(vsb[:, st, :, 0:64], pv.rearrange(
                    "p (h d) -> p h d", h=6))
            return emit

        def dw_ct(ct, sb):
            def emit():
                pdw = pj.tile([128, 512], dt.float32, tag="pj", name="psd")
                for kp in range(4):      # tap pairs (0,1)..(6,7), DoubleRow
                    base = xl8[:, ct, 2 * kp + sb * 512: 2 * kp + sb * 512 + 1]
                    rhs = bass.AP(
                        tensor=xl8.tensor, offset=base.offset,
                        ap=[list(base.ap[0]), [1, 2], [1, 512]])
                    nc.tensor.matmul(
                        pdw, dwd_sb[:, ct, kp, :].rearrange(
                            "p (two w) -> p two w", two=2), rhs,
                        start=(kp == 0), stop=False, perf_mode=DR)
                nc.tensor.matmul(      # tap 8, plain fp8
                    pdw, dwd_sb[:, ct, 4, 0:128],
                    xl8[:, ct, 8 + sb * 512: 8 + sb * 512 + 512],
                    start=False, stop=True)
                nc.vector.tensor_copy(dwT[:, ct, sb * 512:(sb + 1) * 512], pdw)
            return emit

        def pw_at(at, sb):
            def emit():
                pp = pj.tile([128, 512], dt.float32, tag="pj", name="psp")
                for dp in range(3):      # ct pairs, DoubleRow
                    nc.tensor.matmul(
                        pp, wpw_sb[:, 2 * dp:2 * dp + 2,
                                   at * 128:(at + 1) * 128],
                        dwT[:, 2 * dp:2 * dp + 2, sb * 512:(sb + 1) * 512],
                        start=(dp == 0), stop=(dp == 2), perf_mode=DR)
                # psum = 1024*kc ; evac to true-scale kc + conv bias
                nc.vector.tensor_scalar(
                    out=kcT[:, at, sb * 512:(sb + 1) * 512], in0=pp,
                    scalar1=1.0 / 1024.0, scalar2=convb_sb[:, at:at + 1],
                    op0=Alu.mult, op1=Alu.add)
            return emit

        def co_at(at, blk):
            def emit():
                o, w = blk
                pco = pj.tile([128, 512], dt.float32, tag="pj", name="psc")
                for dh in range(6):
                    nc.tensor.matmul(
                        pco[:, :w], wco_sb[:, dh, at * 128:(at + 1) * 128],
                        x0[:, dh, 4 + o: 4 + o + w],
                        start=(dh == 0), stop=(dh == 5))
                nc.vector.scalar_tensor_tensor(
                    out=coT[:, at, o:o + w], in0=pco[:, :w],
                    scalar=bco_sb[:, at:at + 1], in1=mask_sb[:, o:o + w],
                    op0=Alu.add, op1=Alu.mult)
            return emit

        def ca_at(at):
            def emit():
                nc.vector.scalar_tensor_tensor(
                    out=caT[:, at, :], in0=kcT[:, at, :], scalar=CASCALE,
                    in1=qT[:, at, :], op0=Alu.mult, op1=Alu.mult)
            return emit

        def ckl_sb(sb):
            def emit():
                pkf = pj.tile([128, 512], dt.float32, tag="pj", name="psl")
                pk = pkf[0:54, :]
                nc.tensor.matmul(
                    pk, wck_sb[:, 0:2, 0:54],
                    caT[:, 0:2, sb * 512:(sb + 1) * 512],
                    start=True, stop=False, perf_mode=DR)
                nc.tensor.matmul(
                    pk, wck_sb[:, 2, 0:54],
                    caT[:, 2, sb * 512:(sb + 1) * 512],
                    start=False, stop=True)
                # psum = W8SCALE*CASCALE * ckl
                nc.scalar.activation(pck[:, sb * 512:(sb + 1) * 512], pk,
                                     Act.Exp, bias=bck_sb,
                                     scale=1.0 / (W8SCALE * CASCALE))
            return emit

        def pck_out():
            def emit():
                nc.scalar.dma_start(out=pck_dram, in_=pck)
            return emit

        # conv window einsum, one tap per filler.  The pck->head broadcast is
        # a 0/1 selector matmul on the PE (sel_sb[:, k, p] = 1 iff pck row
        # 18*at(p)+9*hh(p)+k drives partition p) -- no stride-0 DMA storm.
        def einsum_k(k):
            def emit():
                ckb = work.tile([128, 3, CHUNK], dt.bfloat16, tag="ckb",
                                bufs=2, name="ckb")
                for at in range(3):
                    pb = pj.tile([128, 512], dt.float32, tag="pj", name="psb")
                    for sb in range(2):
                        if sb:
                            pb = pj.tile([128, 512], dt.float32, tag="pj",
                                         name="psb")
                        nc.tensor.matmul(
                            pb, sel_sb[:, 3 * k + at, :],
                            pck[:, sb * 512:(sb + 1) * 512],
                            start=True, stop=True)
                        nc.vector.tensor_copy(
                            ckb[:, at, sb * 512:(sb + 1) * 512], pb)
                cob = coT[:, 0, k:k + 1]
                cosrc = bass.AP(
                    tensor=coT.tensor, offset=cob.offset,
                    ap=[list(cob.ap[0]), [1032, 3], [1, CHUNK]])
                if k == 0:
                    nc.gpsimd.tensor_mul(acc3, ckb, cosrc)
                else:
                    tmp = work.tile([128, 3, CHUNK], dt.bfloat16, tag="tmp",
                                    bufs=2, name="tmp")
                    nc.gpsimd.tensor_mul(tmp, ckb, cosrc)
                    nc.vector.tensor_add(acc3, acc3, tmp)
                if k == K - 1:
                    nc.gpsimd.dma_start(out=out_conv, in_=acc3)
            return emit

        fillers = []
        fillers += [v_st(st) for st in range(16)]
        fillers += [q_at(1), k_at(1, 0), k_at(1, 1), k_at(1, 2), k_at(1, 3)]
        fillers += [dw_ct(ct, sb) for ct in range(6) for sb in range(2)]
        fillers += [q_at(2)]
        fillers += [pw_at(at, sb) for at in range(3) for sb in range(2)]
        fillers += [co_at(at, blk) for at in range(3)
                    for blk in ((0, 512), (512, 512), (1024, 8))]
        fillers += [ca_at(at) for at in range(3)]
        fillers += [ckl_sb(sb) for sb in range(2)]
        fillers += [pck_out()]
        fillers += [einsum_k(k) for k in range(K)]
        fillers += [k_at(2, sb) for sb in range(4)]

        # ---------------- attention: flat one-step software pipeline -------
        # PE order per step i: scores(i), [fillers], ctx(i-1).  ctx(i-1)
        # waits on exp(i-1); scores(i)+filler keep the PE fed while ACT runs,
        # and exp(i) starts the moment exp(i-1) retires.
        q_at(0)()
        for sb in range(4):
            k_at(0, sb)()
        fillers.pop(0)()          # v_st(0) ahead of ctx(h0, 0)

        steps = [(h, sk) for h in range(N_HEADS) for sk in range(16)]
        cps_of = {}
        prev = None               # (h, sk, pt) awaiting its ctx matmuls
        n_fill0 = len(fillers)
        fill_done = 0

        def emit_ctx(h, sk, pt):
            for sb in range(2):
                nc.tensor.matmul(
                    cps_of[h][sb], vsb[:, sk, h, :],
                    pt[:, sb * 512:(sb + 1) * 512],
                    start=(sk == 0), stop=(sk == 15))
            if sk == 15:
                for sb in range(2):
                    cstg = work.tile([65, 512], dt.float32, tag="cstg",
                                     bufs=4, name="cstg")
                    nc.vector.tensor_copy(cstg, cps_of[h][sb])
                    nc.scalar.dma_start(
                        out=out_attn[:, h * CHUNK + sb * 512:
                                     h * CHUNK + (sb + 1) * 512],
                        in_=cstg)

        for i, (h, sk) in enumerate(steps):
            at, lo = h // 2, (h % 2) * 64
            if sk == 0:
                cps_of[h] = [pc.tile([65, 512], dt.float32, tag=f"ctx{sb}",
                                     name=f"cps{sb}") for sb in range(2)]
            sc = pa.tile([128, 1024], dt.float32, tag="sc", name="sc")
            for sb in range(2):
                nc.tensor.matmul(
                    sc[:, sb * 512:(sb + 1) * 512],
                    kT[lo:lo + 64, at, sk * 128:(sk + 1) * 128],
                    qT[lo:lo + 64, at, sb * 512:(sb + 1) * 512],
                    start=True, stop=True)
            # pace fillers: one per step while v tiles stream (steps 0-15),
            # then spread the rest to finish around step 64
            target = i + 2 if i < 16 else 18 + int(
                (i - 15) * (n_fill0 - 18) / 48.0)
            while fill_done < min(target, n_fill0) and fillers:
                fillers.pop(0)()
                fill_done += 1
            pt = work.tile([128, 1024], dt.bfloat16, tag="pt", bufs=3,
                           name="pt")
            nc.scalar.activation(pt, sc, Act.Exp, scale=0.125)
            if prev is not None:
                emit_ctx(*prev)
            prev = (h, sk, pt)
        emit_ctx(*prev)

    nc.compile()
    return nc


def _prep_in_maps(inputs):
    x = np.asarray(inputs["x"], np.float32)
    dw = np.asarray(inputs["dw"], np.float32).reshape(HIDDEN, K)

    def sb_layout(wT, ntile):  # [ntile*128, F] -> [128, ntile*F]
        f = wT.shape[1]
        return np.ascontiguousarray(
            wT.reshape(ntile, 128, f).transpose(1, 0, 2).reshape(128, ntile * f))

    def wprep(w, dtype=BF16, scale=1.0):  # [A, HIDDEN] -> [128, 6*A]
        return sb_layout(np.ascontiguousarray(w.T * scale).astype(dtype), 6)

    megaA = np.zeros((128, A_COLS), BF16)
    megaA[:, A_WQ:A_WQ + 2304] = wprep(inputs["Wq"])
    megaA[:, A_WK:A_WK + 2304] = wprep(inputs["Wk"])

    megaB = np.zeros((128, B_COLS), BF16)
    megaB[:, B_WV:B_WV + 2304] = wprep(inputs["Wv"])
    megaB[:, B_WO:B_WO + 2304] = wprep(inputs["Wco"])
    megaB[0, B_BV:B_BV + ALL_HEAD] = inputs["bv"].astype(BF16)
    # span-weight selector matrices: sel[r, 3k+at, p] = 1 iff
    # r == 18*at + 9*(p//64) + k
    sel = np.zeros((128, 27, 128), BF16)
    for at in range(3):
        for k in range(K):
            for hh in range(2):
                r = 18 * at + 9 * hh + k
                sel[r, 3 * k + at, hh * 64:(hh + 1) * 64] = 1
    megaB[:, B_SEL:] = sel.reshape(128, 27 * 128)

    mega8 = np.zeros((128, F8_COLS), FP8)
    mega8[:, F8_PW:F8_PW + 2304] = wprep(inputs["pw"], FP8, W8SCALE)
    mega8[:, F8_CK:F8_CK + 192] = sb_layout(np.pad(
        np.ascontiguousarray(inputs["Wck"].T * W8SCALE),
        ((0, 0), (0, 10))).astype(FP8), 3)
    # diagonal depthwise matrices (x32): [128, 6ct, 5kp, 2, 128]
    dwdm = np.zeros((128, 6, 5, 2, 128), FP8)
    ii = np.arange(128)
    for ct in range(6):
        for k in range(K):
            dwdm[ii, ct, k // 2, k % 2, ii] = (
                dw[ct * 128 + ii, k] * W8SCALE).astype(FP8)
    mega8[:, F8_DW:] = dwdm.reshape(128, 7680)

    megaF = np.zeros((128, F_COLS), np.float32)
    megaF[:, F_BQ:F_BQ + 3] = np.ascontiguousarray(
        inputs["bq"].reshape(3, 128).T)
    megaF[:, F_BK:F_BK + 3] = np.ascontiguousarray(
        inputs["bk"].reshape(3, 128).T)
    megaF[:, F_CB:F_CB + 3] = np.ascontiguousarray(
        inputs["conv_bias"].reshape(3, 128).T)
    megaF[:, F_BO:F_BO + 3] = np.ascontiguousarray(
        inputs["bco"].reshape(3, 128).T)
    megaF[0:54, F_BCK] = inputs["bck"].astype(np.float32)

    in_maps = []
    for b in range(B):
        xTb = np.ascontiguousarray(x[b].T)          # [768, S] fp32
        for j in range(2):
            g0 = j * CHUNK - 8
            # x0: rotated cols [0,1036) = global rows [g0, g0+1036), OOB->0
            # x1: rotated cols [1036,2060) mod S (all valid rows)
            idx0 = np.arange(g0, g0 + 1036)
            x0 = np.where((idx0 >= 0) & (idx0 < S), xTb[:, idx0 % S], 0.0)
            idx1 = np.arange(g0 + 1032, g0 + 2056) % S   # complement of chunk
            x1 = xTb[:, idx1]
            # comask over co rows o in [0,1032): global row g0+4+o valid
            mrows = np.arange(g0 + 4, g0 + 4 + 1032)
            mB = megaB.copy()
            mB[:, B_MASK:B_MASK + 1032] = (
                (mrows >= 0) & (mrows < S)).astype(BF16).reshape(1, 1032)
            m = {
                "x0": sb_layout(x0.astype(BF16), 6),
                "x1": sb_layout(x1.astype(BF16), 6),
                "megaA": megaA, "megaB": mB, "mega8": mega8, "megaF": megaF,
            }
            in_maps.append(m)
    return in_maps


def _gather_core(r):
    # attention: [65, 6*1024] fp32, row 64 = softmax denominator
    att = np.asarray(r["out_attn"], np.float32).reshape(65, 6, CHUNK)
    ctx = att[0:64] / att[64:65]                       # [64, 6, s]
    ctx = ctx.transpose(2, 1, 0).reshape(CHUNK, ALL_HEAD)
    # conv: [128, 3*1024] bf16 numerators / pck-sum denominators
    cnv = np.asarray(r["out_conv"], np.float32).reshape(128, 3, CHUNK)
    cnv = cnv.transpose(1, 0, 2).reshape(ALL_HEAD, CHUNK)  # [a, s]
    pck = np.asarray(r["pck_out"], np.float32).reshape(6, K, CHUNK)
    den = pck.sum(axis=1)                              # [h, s]
    cnv = cnv.reshape(N_HEADS, HEAD_DIM, CHUNK) / den[:, None, :]
    cnv = cnv.reshape(ALL_HEAD, CHUNK).T               # [s, a]
    return np.concatenate([ctx, cnv], axis=1)          # [1024, 768]


def _gather(results):
    outs = [_gather_core(r) for r in results]
    full = np.stack(outs).reshape(B, 2, CHUNK, 768).reshape(B, S, 768)
    return np.ascontiguousarray(full, np.float32)


def kernel(**inputs):
    from concourse.bass_utils import run_bass_kernel_spmd

    key = "prog"
    if key not in _COMPILED:
        _COMPILED[key] = _build_program()
    nc = _COMPILED[key]
    in_maps = _prep_in_maps(inputs)
    res = run_bass_kernel_spmd(nc, in_maps, list(range(N_CORES)))
    return _gather(res.results)


if __name__ == "__main__":
    import reference
    inp = {k: np.asarray(v) for k, v in reference.setup_inputs().items()}
    got = kernel(**inp)
    want = np.asarray(reference.reference(**inp))
    err = np.linalg.norm(got - want) / np.linalg.norm(want)
    print("rel err:", err)
